# revision 7
# baseline (speedup 1.0000x reference)
"""Joint bilateral filter (5x5) Trainium2 Bass kernel, 8-core data parallel.

coeff = clip(1 - |-0.125 - 50*d|, 0, 1) = relu(0.875 - 50*d),
d = sum_c (t_c - t_c_shift)^2.

Symmetric-tap scheme: coefficient field C_tau on an extended halo domain
serves tap +tau (aligned read) and tap -tau (shifted read).  Partition
(row) shifts are realized by (a) row-offset DMA loads of the slab and
(b) banded-identity matmuls on the tensor engine accumulating num/den in
PSUM.  Column shifts are plain element offsets (odd offsets run the DVE
in 1x mode; exec is hidden under host<->device transfer anyway).

Host interface: ONE fused uint8 wire tensor per core, w[186, 7, 1288]:
channels 0-3 carry the flow v as raw fp16 bytes (bitcast on device),
channels 4-6 carry the guide t quantized to uint8 (cast-DMA'd to fp16 on
device; values 0..255 are exact in fp16 and the 1/255 rescale is folded
into the squared-difference activation scale).  This more than halves
host->device bytes vs all-fp16.  Identity/band matrices are inline Const
tensors baked into the NEFF.  The compiled executable is built once and
cached; steady-state calls only transfer input, execute, fetch output.
"""
import os
import sys

sys.path.insert(0, "/opt/trn_rl_repo")
os.environ.setdefault("JAX_PLATFORMS", "axon,cpu")

import numpy as np

N, C, H, W = 2, 3, 720, 1280
CV = 2
WCH = 7              # wire channels: 0..3 = v fp16-bytes, 4..6 = t uint8
RPC = 180            # output rows per core
PADW = W + 8         # +-4 col zero pad
SLABR = 186          # slab rows: image rows h0-2 .. h0+183 (tail zero-padded)
SQI = float(np.sqrt(50.0) / 255.0)

# 12 unique taps (ty, tx): ty in 0..2, tx in -2..2, upper half only
TAPS = [(ty, tx) for ty in range(3) for tx in range(-2, 3) if ty > 0 or tx > 0]

_STATE = {}


def _band(shift, scale=1.0):
    return (np.eye(128, 128, k=shift) * scale).astype(np.float16)


def _build_nc():
    import concourse.bacc as bacc
    import concourse.mybir as mybir
    from concourse.tile import TileContext

    u8 = mybir.dt.uint8
    fp16 = mybir.dt.float16
    fp32 = mybir.dt.float32

    nc = bacc.Bacc("TRN2", target_bir_lowering=False, debug=False)

    win = nc.dram_tensor("w", [SLABR, WCH, PADW], u8, kind="ExternalInput")
    bds = {nm: nc.inline_tensor(_band(*a), name=nm)
           for nm, a in (("b0", (0,)), ("b1", (1,)), ("b2", (2,)),
                         ("b0c", (0, 0.875)))}
    zcol = nc.inline_tensor(np.zeros((128, 1), np.float16), name="zcol")
    bcol = nc.inline_tensor(np.full((128, 1), 0.875, np.float16), name="bcol")
    out = nc.dram_tensor("out", [RPC, CV, W], fp16, kind="ExternalOutput")

    RELU = mybir.ActivationFunctionType.Relu
    SQUARE = mybir.ActivationFunctionType.Square
    COPY = mybir.ActivationFunctionType.Copy
    ADD = mybir.AluOpType.add
    MULT = mybir.AluOpType.mult
    SUB = mybir.AluOpType.subtract

    with TileContext(nc) as tc:
        with (
            tc.tile_pool(name="const", bufs=1) as cpool,
            tc.tile_pool(name="io", bufs=1) as iop,
            tc.tile_pool(name="work", bufs=3) as wp,
            tc.tile_pool(name="fin", bufs=2) as fp,
            tc.tile_pool(name="psum", bufs=1, space="PSUM") as pp,
        ):
            Bt = {}
            for nm, dram in bds.items():
                t = cpool.tile([128, 128], fp16, tag=nm)
                nc.sync.dma_start(out=t[:], in_=dram[:])
                Bt[nm] = t
            zero16 = cpool.tile([128, 1], fp16, tag="zero16")
            nc.sync.dma_start(out=zero16[:], in_=zcol[:])
            b875 = cpool.tile([128, 1], fp16, tag="b875")
            nc.sync.dma_start(out=b875[:], in_=bcol[:])

            def load_tile_A():
                # T[s]/V[s]: slab rows s..s+127 (row shift via DMA offset).
                T, V = {}, {}
                for s in range(3):
                    tt = iop.tile([128, C, PADW], fp16, tag=f"te{s}")
                    nc.gpsimd.dma_start(out=tt[:],
                                        in_=win[s:s + 128, 4:7, :])
                    T[s] = tt
                    vv = iop.tile([128, CV, PADW], fp16, tag=f"ve{s}")
                    nc.sync.dma_start(out=vv[:],
                                      in_=win[s:s + 128, 0:4, :].bitcast(fp16))
                    V[s] = vv
                return T, V

            def load_tails():
                # slab rows 124..185, full width, for the B-pass stacking
                tt = iop.tile([62, C, PADW], fp16, tag="ttail")
                nc.gpsimd.dma_start(out=tt[:], in_=win[124:186, 4:7, :])
                vt = iop.tile([62, CV, PADW], fp16, tag="vtail")
                nc.sync.dma_start(out=vt[:],
                                  in_=win[124:186, 0:4, :].bitcast(fp16))
                return tt, vt

            def load_tile_B(ttail, vtail):
                # 120 partitions = two 60-row column chunks stacked:
                # chunk1 = slab rows 124+s..183+s cols 0:648,
                # chunk2 = same rows cols 640:1288.  SBUF->SBUF DMA from the
                # tail tiles (partition remap).
                T, V = {}, {}
                for s in range(3):
                    tt = iop.tile([120, C, 648], fp16, tag=f"tb{s}")
                    nc.sync.dma_start(out=tt[0:60, :, :],
                                      in_=ttail[s:s + 60, :, 0:648])
                    nc.sync.dma_start(out=tt[60:120, :, :],
                                      in_=ttail[s:s + 60, :, 640:1288])
                    T[s] = tt
                    vv = iop.tile([120, CV, 648], fp16, tag=f"vb{s}")
                    nc.sync.dma_start(out=vv[0:60, :, :],
                                      in_=vtail[s:s + 60, :, 0:648])
                    nc.sync.dma_start(out=vv[60:120, :, :],
                                      in_=vtail[s:s + 60, :, 640:1288])
                    V[s] = vv
                return T, V

            def do_pass(T, V, P, b, out_specs):
                """One 640-col pass.  P partitions; C-domain = rows [0, PC);
                psum row i is output row i-2 for i in [2, P-2).  b: col base."""
                PC = P - 2
                pnum0 = pp.tile([128, 640], fp32, tag="pnum0")
                pnum1 = pp.tile([128, 640], fp32, tag="pnum1")
                pden = pp.tile([128, 640], fp32, tag="pden")
                pnums = (pnum0, pnum1)
                total = {"n": 25, "d": 24}
                cnt = {}

                def mm(ptile, key, s, n_, lhsT, kk, rhs_ap):
                    i = cnt.get((key, s), 0)
                    cnt[(key, s)] = i + 1
                    tot = total[key[0]]
                    nc.tensor.matmul(
                        out=ptile[0:P, s:s + n_],
                        lhsT=lhsT[0:kk, 0:P],
                        rhs=rhs_ap,
                        start=(i == 0),
                        stop=(i == tot - 1),
                    )

                SL = ((0, 512), (512, 128))
                for (ty, tx) in TAPS:
                    Bs = Bt["b%d" % ty]
                    c1 = b + 2 + tx
                    u0 = b + 4 + tx
                    d_t = wp.tile([128, C, 644], fp16, tag="delta")
                    nc.vector.tensor_tensor(
                        d_t[0:PC, :, :],
                        T[0][0:PC, :, b + 2:b + 2 + 644],
                        T[ty][0:PC, :, c1:c1 + 644],
                        SUB,
                    )
                    s_t = wp.tile([128, C, 644], fp16, tag="sq")
                    nc.scalar.activation(s_t[0:PC, :, :], d_t[0:PC, :, :], SQUARE,
                                         bias=zero16[0:PC, :], scale=SQI)
                    z_t = wp.tile([128, 644], fp16, tag="z")
                    nc.vector.tensor_tensor(z_t[0:PC, :], s_t[0:PC, 0, :],
                                            s_t[0:PC, 1, :], ADD)
                    nc.vector.tensor_tensor(z_t[0:PC, :], z_t[0:PC, :],
                                            s_t[0:PC, 2, :], ADD)
                    c_t = wp.tile([128, 644], fp16, tag="coef")
                    nc.scalar.activation(c_t[0:PC, :], z_t[0:PC, :], RELU,
                                         bias=b875[0:PC, :], scale=-1.0)
                    # products: mw[q] = C[q]*V[q+ty](col+tx); m[q] = C[q]*V[q]
                    mw_t = wp.tile([128, CV, 640], fp16, tag="mw")
                    m_t = wp.tile([128, CV, 644], fp16, tag="m")
                    for c in range(CV):
                        nc.vector.tensor_tensor(
                            mw_t[0:PC, c, :], c_t[0:PC, 2:642],
                            V[ty][0:PC, c, u0:u0 + 640], MULT)
                        nc.vector.tensor_tensor(
                            m_t[0:PC, c, :], c_t[0:PC, :],
                            V[0][0:PC, c, b + 2:b + 2 + 644], MULT)
                    for s, n_ in SL:
                        for c in range(CV):
                            mm(pnums[c], ("n", c), s, n_, Bt["b0"], PC,
                               mw_t[0:PC, c, s:s + n_])
                        mm(pden, ("d",), s, n_, Bt["b0"], PC,
                           c_t[0:PC, s + 2:s + 2 + n_])
                    for s, n_ in SL:
                        for c in range(CV):
                            mm(pnums[c], ("n", c), s, n_, Bs, PC,
                               m_t[0:PC, c, s - tx + 2:s - tx + 2 + n_])
                        mm(pden, ("d",), s, n_, Bs, PC,
                           c_t[0:PC, s - tx + 2:s - tx + 2 + n_])
                # center tap: num += 0.875 * v
                for s, n_ in SL:
                    for c in range(CV):
                        mm(pnums[c], ("n", c), s, n_, Bt["b0c"], PC,
                           V[0][0:PC, c, b + 4 + s:b + 4 + s + n_])
                # finalize on rows [0, PC)
                den_s = fp.tile([128, 640], fp32, tag="den_s")
                nc.vector.tensor_scalar_add(den_s[0:PC, :], pden[0:PC, :], 0.875)
                r32 = fp.tile([128, 640], fp32, tag="r32")
                nc.vector.reciprocal_approx_fast(out=r32[0:PC, :],
                                                 in_=den_s[0:PC, :])
                r16 = fp.tile([128, 640], fp16, tag="r16")
                nc.vector.tensor_copy(r16[0:PC, :], r32[0:PC, :])
                n16 = fp.tile([128, CV, 640], fp16, tag="n16")
                for c in range(CV):
                    nc.scalar.activation(n16[0:PC, c, :], pnums[c][0:PC, :], COPY)
                o_t = fp.tile([128, CV, 640], fp16, tag="o")
                for c in range(CV):
                    nc.vector.tensor_tensor(o_t[0:PC, c, :], n16[0:PC, c, :],
                                            r16[0:PC, :], MULT)
                for (p0, p1, r0, col0) in out_specs:
                    nc.sync.dma_start(
                        out=out[r0:r0 + (p1 - p0), :, col0:col0 + 640],
                        in_=o_t[p0:p1, :, :])

            T, V = load_tile_A()
            ttail, vtail = load_tails()
            do_pass(T, V, 128, 0, [(2, 126, 0, 0)])
            do_pass(T, V, 128, 640, [(2, 126, 0, 640)])
            T, V = load_tile_B(ttail, vtail)
            do_pass(T, V, 120, 0, [(2, 58, 124, 0), (62, 118, 124, 640)])

    nc.compile()
    return nc


def _get_call():
    """Build the Bass module and a cached, compiled 8-core sharded callable.

    Mirrors bass2jax.run_bass_via_pjrt's shard_map construction, but the
    jitted function is created ONCE so steady-state calls skip retracing,
    re-lowering (BIR zstd), NEFF-cache lookups and executable reloads.
    Outputs are fully written by the kernel, so no zero-init donation
    buffers are shipped.
    """
    if "call" in _STATE:
        return _STATE["call"]

    import jax
    from jax.experimental.shard_map import shard_map
    from jax.sharding import Mesh, PartitionSpec

    import concourse.mybir as mybir
    from concourse.bass2jax import (
        _bass_exec_p,
        install_neuronx_cc_hook,
        partition_id_tensor,
    )

    install_neuronx_cc_hook()
    nc = _build_nc()

    partition_name = nc.partition_id_tensor.name if nc.partition_id_tensor else None
    in_names, out_names, out_avals = [], [], []
    for alloc in nc.m.functions[0].allocations:
        if not isinstance(alloc, mybir.MemoryLocationSet):
            continue
        name = alloc.memorylocations[0].name
        if alloc.kind == "ExternalInput":
            if name != partition_name:
                in_names.append(name)
        elif alloc.kind == "ExternalOutput":
            out_names.append(name)
            out_avals.append(jax.core.ShapedArray(
                tuple(alloc.tensor_shape), mybir.dt.np(alloc.dtype)))
    assert in_names == ["w"] and out_names == ["out"], (in_names, out_names)
    if partition_name is not None:
        in_names.append(partition_name)

    def _body(x):
        operands = [x]
        if partition_name is not None:
            operands.append(partition_id_tensor())
        outs = _bass_exec_p.bind(
            *operands,
            out_avals=tuple(out_avals),
            in_names=tuple(in_names),
            out_names=tuple(out_names),
            lowering_input_output_aliases=(),
            sim_require_finite=True,
            sim_require_nnan=True,
            nc=nc,
        )
        return outs[0]

    devices = jax.devices()[:8]
    assert len(devices) == 8, f"need 8 cores, have {len(jax.devices())}"
    mesh = Mesh(np.asarray(devices), ("core",))
    call = jax.jit(shard_map(
        _body, mesh=mesh, in_specs=(PartitionSpec("core"),),
        out_specs=PartitionSpec("core"), check_rep=False))

    # Warm up: compile NEFF + load executable once, outside timed calls.
    np.asarray(call(np.zeros((8 * SLABR, WCH, PADW), np.uint8)))
    _STATE["call"] = call
    return call


def prepare_inputs(t, vector_curr):
    """Pack full inputs into one global uint8 wire array [8*186, 7, 1288]:
    per core 186 rows (image rows h0-2..h0+181 valid, rest zero), channels
    0-3 = v as raw fp16 bytes, 4-6 = t quantized to uint8, columns 4..1283
    valid."""
    t8 = np.rint(np.asarray(t, np.float32) * np.float32(255.0)).astype(np.uint8)
    v16 = np.asarray(vector_curr).astype(np.float16)
    big = np.zeros((8 * SLABR, WCH, PADW), np.uint8)
    for core in range(8):
        n, q = core // 4, core % 4
        h0 = q * RPC
        r0, r1 = h0 - 2, h0 + RPC + 2
        sr0, sr1 = max(r0, 0), min(r1, H)
        nr = sr1 - sr0
        d0 = core * SLABR + (sr0 - r0)
        vs = np.zeros((nr, CV, PADW), np.float16)
        vs[:, :, 4:4 + W] = v16[n, :, sr0:sr1, :].transpose(1, 0, 2)
        big[d0:d0 + nr, 0:4, :] = \
            vs.reshape(nr, -1).view(np.uint8).reshape(nr, 4, PADW)
        big[d0:d0 + nr, 4:7, 4:4 + W] = t8[n, :, sr0:sr1, :].transpose(1, 0, 2)
    return big


def run_on_device(big):
    """Timed path: host numpy wire array -> device -> execute -> host out.

    Cores finish at staggered times (core i's exec completes right after
    shard i's upload); issuing per-shard async host copies immediately lets
    early shards' D2H overlap later shards' H2D on the full-duplex tunnel.
    """
    call = _get_call()
    r = call(big)
    for s in r.addressable_shards:
        s.data.copy_to_host_async()
    return np.asarray(r)


def kernel(t, vector_curr, **_unused):
    big = prepare_inputs(t, vector_curr)
    res = run_on_device(big).reshape(8, RPC, CV, W)
    outp = np.empty((N, CV, H, W), np.float16)
    for core in range(8):
        n, q = core // 4, core % 4
        h0 = q * RPC
        outp[n, :, h0:h0 + RPC, :] = res[core].transpose(1, 0, 2)
    return outp


# revision 8
# speedup vs baseline: 1.3724x; 1.3724x over previous
"""Joint bilateral filter (5x5) Trainium2 Bass kernel, 8-core data parallel.

coeff = clip(1 - |-0.125 - 50*d|, 0, 1) = relu(0.875 - 50*d),
d = sum_c (t_c - t_c_shift)^2.

Symmetric-tap scheme: coefficient field C_tau on an extended halo domain
serves tap +tau (aligned read) and tap -tau (shifted read).  Partition
(row) shifts are realized by (a) row-offset DMA loads of the slab and
(b) banded-identity matmuls on the tensor engine accumulating num/den in
PSUM.  Column shifts are plain element offsets (odd offsets run the DVE
in 1x mode; exec is hidden under host<->device transfer anyway).

Host interface: ONE fused uint8 wire tensor per core, w[186, 7, 1288]:
channels 0-3 carry the flow v as raw fp16 bytes (bitcast on device),
channels 4-6 carry the guide t quantized to uint8 (cast-DMA'd to fp16 on
device; values 0..255 are exact in fp16 and the 1/255 rescale is folded
into the squared-difference activation scale).  This more than halves
host->device bytes vs all-fp16.  Identity/band matrices are inline Const
tensors baked into the NEFF.  The compiled executable is built once and
cached; steady-state calls only transfer input, execute, fetch output.
"""
import os
import sys

sys.path.insert(0, "/opt/trn_rl_repo")
os.environ.setdefault("JAX_PLATFORMS", "axon,cpu")

import numpy as np

N, C, H, W = 2, 3, 720, 1280
CV = 2
WCH = 7              # wire channels: 0..3 = v fp16-bytes, 4..6 = t uint8
RPC = 180            # output rows per core
PADW = W + 8         # +-4 col zero pad
SLABR = 186          # slab rows: image rows h0-2 .. h0+183 (tail zero-padded)
SQI = float(np.sqrt(50.0) / 255.0)

# 12 unique taps (ty, tx): ty in 0..2, tx in -2..2, upper half only
TAPS = [(ty, tx) for ty in range(3) for tx in range(-2, 3) if ty > 0 or tx > 0]

_STATE = {}


def _band(shift, scale=1.0):
    return (np.eye(128, 128, k=shift) * scale).astype(np.float16)


def _build_nc():
    import concourse.bacc as bacc
    import concourse.mybir as mybir
    from concourse.tile import TileContext

    u8 = mybir.dt.uint8
    fp16 = mybir.dt.float16
    fp32 = mybir.dt.float32

    nc = bacc.Bacc("TRN2", target_bir_lowering=False, debug=False)

    win = nc.dram_tensor("w", [SLABR, WCH, PADW], u8, kind="ExternalInput")
    bds = {nm: nc.inline_tensor(_band(*a), name=nm)
           for nm, a in (("b0", (0,)), ("b1", (1,)), ("b2", (2,)),
                         ("b0c", (0, 0.875)))}
    zcol = nc.inline_tensor(np.zeros((128, 1), np.float16), name="zcol")
    bcol = nc.inline_tensor(np.full((128, 1), 0.875, np.float16), name="bcol")
    out = nc.dram_tensor("out", [RPC, CV, W], fp16, kind="ExternalOutput")

    RELU = mybir.ActivationFunctionType.Relu
    SQUARE = mybir.ActivationFunctionType.Square
    COPY = mybir.ActivationFunctionType.Copy
    ADD = mybir.AluOpType.add
    MULT = mybir.AluOpType.mult
    SUB = mybir.AluOpType.subtract

    with TileContext(nc) as tc:
        with (
            tc.tile_pool(name="const", bufs=1) as cpool,
            tc.tile_pool(name="io", bufs=1) as iop,
            tc.tile_pool(name="work", bufs=3) as wp,
            tc.tile_pool(name="fin", bufs=2) as fp,
            tc.tile_pool(name="psum", bufs=1, space="PSUM") as pp,
        ):
            Bt = {}
            for nm, dram in bds.items():
                t = cpool.tile([128, 128], fp16, tag=nm)
                nc.sync.dma_start(out=t[:], in_=dram[:])
                Bt[nm] = t
            zero16 = cpool.tile([128, 1], fp16, tag="zero16")
            nc.sync.dma_start(out=zero16[:], in_=zcol[:])
            b875 = cpool.tile([128, 1], fp16, tag="b875")
            nc.sync.dma_start(out=b875[:], in_=bcol[:])

            def load_tile_A():
                # T[s]/V[s]: slab rows s..s+127 (row shift via DMA offset).
                T, V = {}, {}
                for s in range(3):
                    tt = iop.tile([128, C, PADW], fp16, tag=f"te{s}")
                    nc.gpsimd.dma_start(out=tt[:],
                                        in_=win[s:s + 128, 4:7, :])
                    T[s] = tt
                    vv = iop.tile([128, CV, PADW], fp16, tag=f"ve{s}")
                    nc.sync.dma_start(out=vv[:],
                                      in_=win[s:s + 128, 0:4, :].bitcast(fp16))
                    V[s] = vv
                return T, V

            def load_tails():
                # slab rows 124..185, full width, for the B-pass stacking
                tt = iop.tile([62, C, PADW], fp16, tag="ttail")
                nc.gpsimd.dma_start(out=tt[:], in_=win[124:186, 4:7, :])
                vt = iop.tile([62, CV, PADW], fp16, tag="vtail")
                nc.sync.dma_start(out=vt[:],
                                  in_=win[124:186, 0:4, :].bitcast(fp16))
                return tt, vt

            def load_tile_B(ttail, vtail):
                # 120 partitions = two 60-row column chunks stacked:
                # chunk1 = slab rows 124+s..183+s cols 0:648,
                # chunk2 = same rows cols 640:1288.  SBUF->SBUF DMA from the
                # tail tiles (partition remap).
                T, V = {}, {}
                for s in range(3):
                    tt = iop.tile([120, C, 648], fp16, tag=f"tb{s}")
                    nc.sync.dma_start(out=tt[0:60, :, :],
                                      in_=ttail[s:s + 60, :, 0:648])
                    nc.sync.dma_start(out=tt[60:120, :, :],
                                      in_=ttail[s:s + 60, :, 640:1288])
                    T[s] = tt
                    vv = iop.tile([120, CV, 648], fp16, tag=f"vb{s}")
                    nc.sync.dma_start(out=vv[0:60, :, :],
                                      in_=vtail[s:s + 60, :, 0:648])
                    nc.sync.dma_start(out=vv[60:120, :, :],
                                      in_=vtail[s:s + 60, :, 640:1288])
                    V[s] = vv
                return T, V

            def do_pass(T, V, P, b, out_specs):
                """One 640-col pass.  P partitions; C-domain = rows [0, PC);
                psum row i is output row i-2 for i in [2, P-2).  b: col base."""
                PC = P - 2
                pnum0 = pp.tile([128, 640], fp32, tag="pnum0")
                pnum1 = pp.tile([128, 640], fp32, tag="pnum1")
                pden = pp.tile([128, 640], fp32, tag="pden")
                pnums = (pnum0, pnum1)
                total = {"n": 25, "d": 24}
                cnt = {}

                def mm(ptile, key, s, n_, lhsT, kk, rhs_ap):
                    i = cnt.get((key, s), 0)
                    cnt[(key, s)] = i + 1
                    tot = total[key[0]]
                    nc.tensor.matmul(
                        out=ptile[0:P, s:s + n_],
                        lhsT=lhsT[0:kk, 0:P],
                        rhs=rhs_ap,
                        start=(i == 0),
                        stop=(i == tot - 1),
                    )

                SL = ((0, 512), (512, 128))
                for (ty, tx) in TAPS:
                    Bs = Bt["b%d" % ty]
                    c1 = b + 2 + tx
                    u0 = b + 4 + tx
                    d_t = wp.tile([128, C, 644], fp16, tag="delta")
                    nc.vector.tensor_tensor(
                        d_t[0:PC, :, :],
                        T[0][0:PC, :, b + 2:b + 2 + 644],
                        T[ty][0:PC, :, c1:c1 + 644],
                        SUB,
                    )
                    s_t = wp.tile([128, C, 644], fp16, tag="sq")
                    nc.scalar.activation(s_t[0:PC, :, :], d_t[0:PC, :, :], SQUARE,
                                         bias=zero16[0:PC, :], scale=SQI)
                    z_t = wp.tile([128, 644], fp16, tag="z")
                    nc.vector.tensor_tensor(z_t[0:PC, :], s_t[0:PC, 0, :],
                                            s_t[0:PC, 1, :], ADD)
                    nc.vector.tensor_tensor(z_t[0:PC, :], z_t[0:PC, :],
                                            s_t[0:PC, 2, :], ADD)
                    c_t = wp.tile([128, 644], fp16, tag="coef")
                    nc.scalar.activation(c_t[0:PC, :], z_t[0:PC, :], RELU,
                                         bias=b875[0:PC, :], scale=-1.0)
                    # products: mw[q] = C[q]*V[q+ty](col+tx); m[q] = C[q]*V[q]
                    mw_t = wp.tile([128, CV, 640], fp16, tag="mw")
                    m_t = wp.tile([128, CV, 644], fp16, tag="m")
                    for c in range(CV):
                        nc.vector.tensor_tensor(
                            mw_t[0:PC, c, :], c_t[0:PC, 2:642],
                            V[ty][0:PC, c, u0:u0 + 640], MULT)
                        nc.vector.tensor_tensor(
                            m_t[0:PC, c, :], c_t[0:PC, :],
                            V[0][0:PC, c, b + 2:b + 2 + 644], MULT)
                    for s, n_ in SL:
                        for c in range(CV):
                            mm(pnums[c], ("n", c), s, n_, Bt["b0"], PC,
                               mw_t[0:PC, c, s:s + n_])
                        mm(pden, ("d",), s, n_, Bt["b0"], PC,
                           c_t[0:PC, s + 2:s + 2 + n_])
                    for s, n_ in SL:
                        for c in range(CV):
                            mm(pnums[c], ("n", c), s, n_, Bs, PC,
                               m_t[0:PC, c, s - tx + 2:s - tx + 2 + n_])
                        mm(pden, ("d",), s, n_, Bs, PC,
                           c_t[0:PC, s - tx + 2:s - tx + 2 + n_])
                # center tap: num += 0.875 * v
                for s, n_ in SL:
                    for c in range(CV):
                        mm(pnums[c], ("n", c), s, n_, Bt["b0c"], PC,
                           V[0][0:PC, c, b + 4 + s:b + 4 + s + n_])
                # finalize on rows [0, PC)
                den_s = fp.tile([128, 640], fp32, tag="den_s")
                nc.vector.tensor_scalar_add(den_s[0:PC, :], pden[0:PC, :], 0.875)
                r32 = fp.tile([128, 640], fp32, tag="r32")
                nc.vector.reciprocal_approx_fast(out=r32[0:PC, :],
                                                 in_=den_s[0:PC, :])
                r16 = fp.tile([128, 640], fp16, tag="r16")
                nc.vector.tensor_copy(r16[0:PC, :], r32[0:PC, :])
                n16 = fp.tile([128, CV, 640], fp16, tag="n16")
                for c in range(CV):
                    nc.scalar.activation(n16[0:PC, c, :], pnums[c][0:PC, :], COPY)
                o_t = fp.tile([128, CV, 640], fp16, tag="o")
                for c in range(CV):
                    nc.vector.tensor_tensor(o_t[0:PC, c, :], n16[0:PC, c, :],
                                            r16[0:PC, :], MULT)
                for (p0, p1, r0, col0) in out_specs:
                    nc.sync.dma_start(
                        out=out[r0:r0 + (p1 - p0), :, col0:col0 + 640],
                        in_=o_t[p0:p1, :, :])

            T, V = load_tile_A()
            ttail, vtail = load_tails()
            do_pass(T, V, 128, 0, [(2, 126, 0, 0)])
            do_pass(T, V, 128, 640, [(2, 126, 0, 640)])
            T, V = load_tile_B(ttail, vtail)
            do_pass(T, V, 120, 0, [(2, 58, 124, 0), (62, 118, 124, 640)])

    nc.compile()
    return nc


def _get_call():
    """Build the Bass module and a cached, compiled 8-core sharded callable.

    Mirrors bass2jax.run_bass_via_pjrt's shard_map construction, but the
    jitted function is created ONCE so steady-state calls skip retracing,
    re-lowering (BIR zstd), NEFF-cache lookups and executable reloads.
    Outputs are fully written by the kernel, so no zero-init donation
    buffers are shipped.
    """
    if "call" in _STATE:
        return _STATE["call"]

    import jax
    from jax.experimental.shard_map import shard_map
    from jax.sharding import Mesh, PartitionSpec

    import concourse.mybir as mybir
    from concourse.bass2jax import (
        _bass_exec_p,
        install_neuronx_cc_hook,
        partition_id_tensor,
    )

    install_neuronx_cc_hook()
    nc = _build_nc()

    partition_name = nc.partition_id_tensor.name if nc.partition_id_tensor else None
    in_names, out_names, out_avals = [], [], []
    for alloc in nc.m.functions[0].allocations:
        if not isinstance(alloc, mybir.MemoryLocationSet):
            continue
        name = alloc.memorylocations[0].name
        if alloc.kind == "ExternalInput":
            if name != partition_name:
                in_names.append(name)
        elif alloc.kind == "ExternalOutput":
            out_names.append(name)
            out_avals.append(jax.core.ShapedArray(
                tuple(alloc.tensor_shape), mybir.dt.np(alloc.dtype)))
    assert in_names == ["w"] and out_names == ["out"], (in_names, out_names)
    if partition_name is not None:
        in_names.append(partition_name)

    def _body(x):
        operands = [x]
        if partition_name is not None:
            operands.append(partition_id_tensor())
        outs = _bass_exec_p.bind(
            *operands,
            out_avals=tuple(out_avals),
            in_names=tuple(in_names),
            out_names=tuple(out_names),
            lowering_input_output_aliases=(),
            sim_require_finite=True,
            sim_require_nnan=True,
            nc=nc,
        )
        return outs[0]

    devices = jax.devices()[:8]
    assert len(devices) == 8, f"need 8 cores, have {len(jax.devices())}"
    mesh = Mesh(np.asarray(devices), ("core",))
    call = jax.jit(shard_map(
        _body, mesh=mesh, in_specs=(PartitionSpec("core"),),
        out_specs=PartitionSpec("core"), check_rep=False))

    # Warm up: compile NEFF + load executable once, outside timed calls.
    np.asarray(call(np.zeros((8 * SLABR, WCH, PADW), np.uint8)))
    _STATE["call"] = call
    return call


def prepare_inputs(t, vector_curr):
    """Pack full inputs into one global uint8 wire array [8*186, 7, 1288]:
    per core 186 rows (image rows h0-2..h0+181 valid, rest zero), channels
    0-3 = v as raw fp16 bytes, 4-6 = t quantized to uint8, columns 4..1283
    valid."""
    t8 = np.rint(np.asarray(t, np.float32) * np.float32(255.0)).astype(np.uint8)
    v16 = np.asarray(vector_curr).astype(np.float16)
    big = np.zeros((8 * SLABR, WCH, PADW), np.uint8)
    for core in range(8):
        n, q = core // 4, core % 4
        h0 = q * RPC
        r0, r1 = h0 - 2, h0 + RPC + 2
        sr0, sr1 = max(r0, 0), min(r1, H)
        nr = sr1 - sr0
        d0 = core * SLABR + (sr0 - r0)
        vs = np.zeros((nr, CV, PADW), np.float16)
        vs[:, :, 4:4 + W] = v16[n, :, sr0:sr1, :].transpose(1, 0, 2)
        big[d0:d0 + nr, 0:4, :] = \
            vs.reshape(nr, -1).view(np.uint8).reshape(nr, 4, PADW)
        big[d0:d0 + nr, 4:7, 4:4 + W] = t8[n, :, sr0:sr1, :].transpose(1, 0, 2)
    return big


def run_on_device(big):
    """Timed path: host numpy wire array -> device -> execute -> host out."""
    call = _get_call()
    return np.asarray(call(big))


def kernel(t, vector_curr, **_unused):
    big = prepare_inputs(t, vector_curr)
    res = run_on_device(big).reshape(8, RPC, CV, W)
    outp = np.empty((N, CV, H, W), np.float16)
    for core in range(8):
        n, q = core // 4, core % 4
        h0 = q * RPC
        outp[n, :, h0:h0 + RPC, :] = res[core].transpose(1, 0, 2)
    return outp
